# revision 21
# baseline (speedup 1.0000x reference)
"""KBLAM Llama attention (sparse KB top-k) on 8 Trainium2 cores.

Strategy (v2, bf16):
- Host: top-k KB selection collapses algebraically to a matvec
  (scores = kb_keys @ (Wq2 @ sum_q hidden)) computed in fp64; gather kb rows;
  shard weights over heads (4 heads / core); transpose hidden once; convert
  hidden/weights/KB to bf16.
- Device (per core, bf16 matmuls with fp32 PSUM accumulation), per 512-query
  super-block qs:
    R1: K^T projection (4 PSUM banks) + rope -> kall bf16, overlapping the
        previous block's attention (4 PSUM banks).
    R2: Q^T projection + rope -> qsb bf16, and V projection directly in
        natural [token, dim] layout (stationary = hidden chunk, moving = Wv)
        -> vall bf16 with no PE transposes.
    A:  blocked attention in S^T layout; exp on Act engine (bf16 out);
        softmax denominator via GpSimd SBUF accumulation (esum) + one
        ones-matmul; PV accumulation in PSUM; normalize with a broadcast
        matmul + DVE multiply into oall bf16.
- Wo phase: row-parallel output projection from oall; partial [4096, 2048]
  fp32 written straight from PSUM; partials summed on the host.
"""
import sys
import numpy as np

sys.path.insert(0, '/opt/trn_rl_repo')

import concourse.bacc as bacc
import concourse.mybir as mybir
from concourse.tile import TileContext
from concourse import bass_utils

H = 32
HD = 128
HID = 4096
KB_LEN = 1024
TOPK = 128
Q = 2048
ROPE_THETA = 10000.0
PAD = -1000000000.0
N_CORES = 8
HPC = H // N_CORES          # heads per core = 4
DPC = HPC * HD              # dout per core = 512
QS = 512                    # query super-block width
NQS = Q // QS               # 4
CC = HID // 128             # 32 contraction chunks
NJB = 1 + Q // 128          # 17 j-blocks (kb + 16 self)
SCALE = 1.0 / np.sqrt(HD)
CCG = 4                     # cc chunks per hT stream tile
NST = CC // CCG             # stream tiles per pass = 8

F32 = mybir.dt.float32
F32R = mybir.dt.float32r
BF16 = mybir.dt.bfloat16
EXP = mybir.ActivationFunctionType.Exp

_CACHED_NC = None


def build():
    nc = bacc.Bacc("TRN2", target_bir_lowering=False, debug=False,
                   num_devices=N_CORES)

    hT = nc.dram_tensor("hT", [HID, Q], BF16, kind="ExternalInput")
    wkT = nc.dram_tensor("wkT", [HID, DPC], BF16, kind="ExternalInput")
    wvT = nc.dram_tensor("wvT", [HID, DPC], BF16, kind="ExternalInput")
    wqT = nc.dram_tensor("wqT", [HID, DPC], BF16, kind="ExternalInput")
    # wo pre-tiled on host: [32(ho), 128(p), HPC(dc), 128] bf16
    woR = nc.dram_tensor("woR", [HID // 128, 128, HPC, 128], BF16,
                         kind="ExternalInput")
    kbkT = nc.dram_tensor("kbkT", [HPC, HD, TOPK], BF16, kind="ExternalInput")
    kbv = nc.dram_tensor("kbv", [TOPK, DPC], BF16, kind="ExternalInput")
    cosT = nc.dram_tensor("cosT", [HD, Q], F32, kind="ExternalInput")
    sinTs = nc.dram_tensor("sinTs", [HD, Q], F32, kind="ExternalInput")  # sign-folded
    tri = nc.dram_tensor("tri", [128, 128], F32, kind="ExternalInput")
    shift = nc.dram_tensor("shift", [128, HPC], F32, kind="ExternalInput")
    onesc = nc.dram_tensor("onesc", [128, 1], F32R, kind="ExternalInput")
    onesr = nc.dram_tensor("onesr", [1, 128], F32R, kind="ExternalInput")
    outT = nc.dram_tensor("outT", [HID, Q], BF16, kind="ExternalOutput")

    with TileContext(nc) as tc:
        with tc.tile_pool(name="const", bufs=1) as p_const, \
             tc.tile_pool(name="wmat", bufs=1) as p_w, \
             tc.tile_pool(name="big", bufs=1) as p_big:
            t_tri = p_const.tile([128, 128], F32)
            t_shift = p_const.tile([128, HPC], F32)
            t_onesc = p_const.tile([128, 1], F32R)
            t_onesr = p_const.tile([1, 128], F32R)

            # resident bf16 weights: [128, CC, DPC] each
            wk_sb = p_w.tile([128, CC, DPC], BF16, tag="wk", name="wk")
            wq_sb = p_w.tile([128, CC, DPC], BF16, tag="wq", name="wq")
            wv_sb = p_w.tile([128, CC, DPC], BF16, tag="wv", name="wv")
            # weight-chunk DMAs are emitted interleaved with the qs=0
            # streaming loops below so the first matmuls aren't queued
            # behind 12 MiB of weight traffic on the DMA engines
            wkT_r = wkT.ap().rearrange("(cc p) d -> p cc d", p=128)
            wqT_r = wqT.ap().rearrange("(cc p) d -> p cc d", p=128)
            wvT_r = wvT.ap().rearrange("(cc p) d -> p cc d", p=128)

            # persistent per-head K^T (bf16), merged V (natural layout, bf16),
            # per-head attention outputs (bf16)
            kall = [p_big.tile([128, NJB, 128], BF16, tag=f"kall{h}",
                               name=f"kall{h}") for h in range(HPC)]
            vall = p_big.tile([128, NJB, DPC], BF16, tag="vall", name="vall")
            oall = [p_big.tile([128, Q], BF16, tag=f"oall{h}",
                               name=f"oall{h}") for h in range(HPC)]

            hT_r = hT.ap().rearrange("(cc p) q -> p cc q", p=128)

            def rope(p_in, cos_s, sin_s, out_ap, tmp_pool):
                # out = p*cos + rot_half(p)*sin  (sin sign-folded on host)
                t1 = tmp_pool.tile([128, QS], F32, tag="rope1")
                t2 = tmp_pool.tile([128, QS], F32, tag="rope2")
                nc.vector.tensor_mul(t1[:], p_in[:], cos_s)
                nc.vector.tensor_mul(t2[0:64], p_in[64:128], sin_s[0:64])
                nc.vector.tensor_mul(t2[64:128], p_in[0:64], sin_s[64:128])
                with nc.allow_low_precision(reason="bf16 store"):
                    nc.vector.tensor_add(out_ap, t1[:], t2[:])

            with tc.tile_pool(name="ht", bufs=3) as p_ht, \
                 tc.tile_pool(name="cs", bufs=2) as p_cs, \
                 tc.tile_pool(name="rope", bufs=2) as p_rope, \
                 tc.tile_pool(name="qsb", bufs=8) as p_q, \
                 tc.tile_pool(name="e", bufs=3) as p_e, \
                 tc.tile_pool(name="esum", bufs=2) as p_es, \
                 tc.tile_pool(name="trec", bufs=2) as p_tr, \
                 tc.tile_pool(name="psP", bufs=4, space="PSUM") as psP, \
                 tc.tile_pool(name="psS", bufs=2, space="PSUM") as psS, \
                 tc.tile_pool(name="psO", bufs=2, space="PSUM") as psO:

                for qs in range(NQS):
                    qsl = slice(qs * QS, (qs + 1) * QS)
                    if qs > 0:
                        cos_s = p_cs.tile([128, QS], F32, tag="cos")
                        sin_s = p_cs.tile([128, QS], F32, tag="sin")
                        nc.sync.dma_start(cos_s[:], cosT.ap()[:, qsl])
                        nc.sync.dma_start(sin_s[:], sinTs.ap()[:, qsl])

                    # ---------------- R1: K^T projection + rope ----------------
                    pk = [psP.tile([128, QS], F32, tag="p", name=f"pk{qs}_{h}")
                          for h in range(HPC)]
                    for g in range(NST):
                        csl = slice(g * CCG, (g + 1) * CCG)
                        if qs == 0:
                            nc.sync.dma_start(wk_sb[:, csl, :], wkT_r[:, csl, :])
                        ht = p_ht.tile([128, CCG, QS], BF16, tag="ht")
                        nc.sync.dma_start(
                            ht[:], hT_r[:, g * CCG:(g + 1) * CCG, qsl])
                        if qs == 0 and g == 0:
                            # rope tables for qs=0 (needed right after R1 to
                            # free the projection PSUM slots)
                            cos_s = p_cs.tile([128, QS], F32, tag="cos")
                            sin_s = p_cs.tile([128, QS], F32, tag="sin")
                            nc.sync.dma_start(cos_s[:], cosT.ap()[:, qsl])
                            nc.sync.dma_start(sin_s[:], sinTs.ap()[:, qsl])
                        for ci in range(CCG):
                            cc = g * CCG + ci
                            for h in range(HPC):
                                nc.tensor.matmul(
                                    pk[h][:], wk_sb[:, cc, h * HD:(h + 1) * HD],
                                    ht[:, ci, :], start=(cc == 0),
                                    stop=(cc == CC - 1))
                    for h in range(HPC):
                        kslice = kall[h][:, 1 + 4 * qs: 5 + 4 * qs, :] \
                            .rearrange("p a b -> p (a b)")
                        rope(pk[h], cos_s[:], sin_s[:], kslice, p_rope)

                    # ---------------- R2: Q^T projection + rope, V natural ----
                    pq = [psP.tile([128, QS], F32, tag="p", name=f"pq{qs}_{h}")
                          for h in range(HPC)]
                    # V accumulators borrow the attention pools' banks, which
                    # are idle during R2 (previous block's attention finished
                    # during R1).
                    pv = [psO.tile([128, QS], F32, tag="po", name=f"pv{qs}_0"),
                          psO.tile([128, QS], F32, tag="po", name=f"pv{qs}_1"),
                          psS.tile([128, QS], F32, tag="s", name=f"pv{qs}_2"),
                          psS.tile([128, QS], F32, tag="s", name=f"pv{qs}_3")]
                    for g in range(NST):
                        csl = slice(g * CCG, (g + 1) * CCG)
                        ht = p_ht.tile([128, CCG, QS], BF16, tag="ht")
                        nc.sync.dma_start(
                            ht[:], hT_r[:, g * CCG:(g + 1) * CCG, qsl])
                        if qs == 0 and g == 0:
                            sl0 = slice(0, CCG)
                            nc.sync.dma_start(wv_sb[:, sl0, :], wvT_r[:, sl0, :])
                            nc.sync.dma_start(wq_sb[:, sl0, :], wqT_r[:, sl0, :])
                        if qs == 0 and g == 1:
                            # attention constants and KB rows (A(0) only)
                            nc.sync.dma_start(t_tri[:], tri.ap())
                            nc.sync.dma_start(t_shift[:], shift.ap())
                            nc.sync.dma_start(t_onesc[:], onesc.ap())
                            nc.sync.dma_start(t_onesr[:], onesr.ap())
                            for h in range(HPC):
                                nc.sync.dma_start(kall[h][:, 0, :], kbkT.ap()[h])
                            nc.sync.dma_start(vall[:, 0, :], kbv.ap())
                        if qs == 0 and g + 1 < NST:
                            pf_sl = slice((g + 1) * CCG, (g + 2) * CCG)
                            nc.sync.dma_start(wv_sb[:, pf_sl, :],
                                              wvT_r[:, pf_sl, :])
                            nc.sync.dma_start(wq_sb[:, pf_sl, :],
                                              wqT_r[:, pf_sl, :])
                        for ci in range(CCG):
                            cc = g * CCG + ci

                            def q_mms():
                                for h in range(HPC):
                                    nc.tensor.matmul(
                                        pq[h][:],
                                        wq_sb[:, cc, h * HD:(h + 1) * HD],
                                        ht[:, ci, :], start=(cc == 0),
                                        stop=(cc == CC - 1))

                            def v_mms():
                                for t in range(4):
                                    nc.tensor.matmul(
                                        pv[t][:],
                                        ht[:, ci, t * 128:(t + 1) * 128],
                                        wv_sb[:, cc, :], start=(cc == 0),
                                        stop=(cc == CC - 1))

                            # first chunk: V before Q so the PE has rope-free
                            # work while rope-K drains the K PSUM slots; later
                            # chunks: Q first so rope-Q overlaps V's tail
                            if g == 0:
                                v_mms(); q_mms()
                            else:
                                q_mms(); v_mms()
                    qsb = []
                    for h in range(HPC):
                        qt = p_q.tile([128, QS], BF16, tag="q",
                                      name=f"q{qs}_{h}")
                        rope(pq[h], cos_s[:], sin_s[:], qt[:], p_rope)
                        qsb.append(qt)
                    for t in range(4):
                        # on Act, not DVE: keeps DVE free for the rope chains
                        # that gate the next phase's PSUM slots
                        nc.scalar.copy(out=vall[:, 1 + 4 * qs + t, :],
                                       in_=pv[t][:])

                    # ---------------- A: attention for this super-block -------
                    njb_self = 4 * (qs + 1)
                    for h in range(HPC):
                        po = psO.tile([128, QS], F32, tag="po",
                                      name=f"po{qs}_{h}")
                        esum = p_es.tile([128, QS], F32R, tag="esum",
                                         name=f"es{qs}_{h}")
                        for jb in range(njb_self + 1):
                            if jb == 0:  # kb block, full width
                                off, n = 0, QS
                            else:
                                bj = jb - 1
                                t = bj - 4 * qs
                                off = max(0, t * 128)
                                n = QS - off
                            ps = psS.tile([128, QS], F32, tag="s")
                            nc.tensor.matmul(ps[:, :n], kall[h][:, jb, :],
                                             qsb[h][:, off:QS],
                                             start=True, stop=True)
                            e = p_e.tile([128, QS], BF16, tag="e")
                            if jb == 0:
                                nc.scalar.activation(e[:, :n], ps[:, :n], EXP,
                                                     bias=t_shift[:, h:h + 1],
                                                     scale=SCALE)
                                nc.gpsimd.tensor_copy(esum[:], e[:])
                            else:
                                if off > 0 or bj == 4 * qs:
                                    # diagonal-crossing block: mask first 128
                                    nc.vector.tensor_add(ps[:, :128],
                                                         ps[:, :128], t_tri[:])
                                nc.scalar.activation(e[:, :n], ps[:, :n], EXP,
                                                     scale=SCALE)
                                nc.gpsimd.tensor_add(esum[:, off:QS],
                                                     esum[:, off:QS], e[:, :n])
                            nc.tensor.matmul(po[:, off:QS],
                                             vall[:, jb, h * HD:(h + 1) * HD],
                                             e[:, :n],
                                             start=(jb == 0),
                                             stop=(jb == njb_self),
                                             skip_group_check=True)
                        pr = psS.tile([128, QS], F32, tag="s")
                        nc.tensor.matmul(pr[0:1, :], t_onesc[:], esum[:],
                                         start=True, stop=True)
                        trec = p_tr.tile([1, QS], F32R, tag="trec")
                        with nc.allow_low_precision(reason="fp32r rounding"):
                            nc.vector.reciprocal(trec[:], pr[0:1, :])
                        # broadcast 1/r to all partitions in SBUF (GpSimd) so
                        # the DVE multiply has only one PSUM operand
                        trb = p_tr.tile([128, QS], F32R, tag="trb")
                        nc.gpsimd.partition_broadcast(trb[:], trec[:])
                        with nc.allow_low_precision(reason="bf16 store"):
                            nc.vector.tensor_mul(oall[h][:, qsl], po[:],
                                                 trb[:])

            # ---------------- Phase W: output projection ----------------
            with tc.tile_pool(name="wo", bufs=3) as p_wo, \
                 tc.tile_pool(name="wout", bufs=3) as p_wout, \
                 tc.tile_pool(name="psW", bufs=4, space="PSUM") as psW:
                for ho in range(HID // 128):
                    wo_t = p_wo.tile([128, HPC, 128], BF16, tag="wo")
                    nc.sync.dma_start(wo_t[:], woR.ap()[ho])
                    for qs in range(NQS):
                        pf = psW.tile([128, QS], F32, tag="pf")
                        for dc in range(HPC):
                            nc.tensor.matmul(
                                pf[:], wo_t[:, dc, :],
                                oall[dc][:, qs * QS:(qs + 1) * QS],
                                start=(dc == 0), stop=(dc == HPC - 1))
                        wout = p_wout.tile([128, QS], BF16, tag="wout")
                        nc.scalar.copy(out=wout[:], in_=pf[:])
                        nc.sync.dma_start(
                            outT.ap()[ho * 128:(ho + 1) * 128,
                                      qs * QS:(qs + 1) * QS],
                            wout[:])
    nc.compile()
    return nc


def _host_prep(hidden_states, position_ids, kb_keys, kb_values,
               Wq, Wq2, Wk, Wv, Wo, score_shift):
    import ml_dtypes
    bf16 = ml_dtypes.bfloat16

    hid = np.asarray(hidden_states, dtype=np.float32)[0]          # [Q, HID]
    # --- top-k KB selection (exact algebraic collapse of reference) ---
    hsum = hid.astype(np.float64).sum(axis=0)                     # [HID]
    q2sum = np.asarray(Wq2, dtype=np.float64) @ hsum              # [HID]
    scores = np.asarray(kb_keys, dtype=np.float64) @ q2sum        # [KB_LEN]
    idx = np.argpartition(-scores, TOPK)[:TOPK]
    kbk_sel = np.asarray(kb_keys, np.float32)[idx]                # [TOPK, HID]
    kbv_sel = np.asarray(kb_values, np.float32)[idx]

    # --- rope tables (transposed layout, sign-folded sin) ---
    pos = np.asarray(position_ids)[0].astype(np.float64)          # [Q]
    inv_freq = 1.0 / (ROPE_THETA ** (np.arange(0, HD, 2, dtype=np.float64) / HD))
    freqs = pos[:, None] * inv_freq                               # [Q, HD/2]
    emb = np.concatenate([freqs, freqs], axis=1)                  # [Q, HD]
    cosT = np.ascontiguousarray(np.cos(emb).T.astype(np.float32))  # [HD, Q]
    sinT = np.sin(emb).T.astype(np.float32)
    sinTs = sinT.copy()
    sinTs[0:64] = -sinT[0:64]
    sinTs = np.ascontiguousarray(sinTs)

    hT = np.ascontiguousarray(hid.T.astype(bf16))                 # [HID, Q] bf16

    tri = np.where(np.arange(128)[None, :] >= np.arange(128)[:, None],
                   0.0, PAD).astype(np.float32)                   # [jl, ql]
    onesc = np.ones((128, 1), np.float32)
    onesr = np.ones((1, 128), np.float32)

    Wq = np.asarray(Wq, np.float32)
    Wk = np.asarray(Wk, np.float32)
    Wv = np.asarray(Wv, np.float32)
    Wo = np.asarray(Wo, np.float32)
    ss = np.asarray(score_shift, np.float32).reshape(H)

    in_maps = []
    for c in range(N_CORES):
        rows = slice(c * DPC, (c + 1) * DPC)
        heads = range(c * HPC, (c + 1) * HPC)
        kbkT = np.stack([np.ascontiguousarray(
            kbk_sel[:, h * HD:(h + 1) * HD].T.astype(bf16)) for h in heads])
        kbv_c = np.ascontiguousarray(kbv_sel[:, rows].astype(bf16))  # [TOPK, DPC]
        shift_c = np.broadcast_to(ss[c * HPC:(c + 1) * HPC][None, :], (128, HPC))
        # wo pre-tiled: [32(ho), 128(p), HPC(dc), 128]
        woT_c = Wo[:, rows].T                                     # [DPC, HID]
        woR = np.ascontiguousarray(
            woT_c.reshape(HPC, 128, HID // 128, 128)
            .transpose(2, 1, 0, 3).astype(bf16))
        in_maps.append({
            "hT": hT,
            "wkT": np.ascontiguousarray(Wk[rows].T.astype(bf16)),
            "wvT": np.ascontiguousarray(Wv[rows].T.astype(bf16)),
            "wqT": np.ascontiguousarray(Wq[rows].T.astype(bf16)),
            "woR": woR,
            "kbkT": kbkT,
            "kbv": kbv_c,
            "cosT": cosT,
            "sinTs": sinTs,
            "tri": tri,
            "shift": np.ascontiguousarray(shift_c),
            "onesc": onesc,
            "onesr": onesr,
        })
    return in_maps


def _numpy_fallback(hidden_states, attention_mask, position_ids, kb_keys, kb_values,
                    Wq, Wq2, Wk, Wv, Wo, score_shift):
    """Pure-numpy reference (only used if the mask is not the expected causal)."""
    B, Qn, _ = hidden_states.shape
    x = np.asarray(hidden_states, np.float32)

    def heads(t):
        return t.reshape(B, Qn, H, HD).transpose(0, 2, 1, 3)

    q = heads(x @ Wq.T)
    q2 = heads(x @ Wq2.T)
    k = heads(x @ Wk.T)
    v = heads(x @ Wv.T)
    pos = np.asarray(position_ids).astype(np.float32)
    inv_freq = 1.0 / (ROPE_THETA ** (np.arange(0, HD, 2, dtype=np.float32) / HD))
    freqs = pos[..., None] * inv_freq
    emb = np.concatenate([freqs, freqs], axis=-1)
    cos, sin = np.cos(emb)[:, None], np.sin(emb)[:, None]

    def rot(t):
        return np.concatenate([-t[..., HD // 2:], t[..., :HD // 2]], axis=-1)

    q, k = q * cos + rot(q) * sin, k * cos + rot(k) * sin
    kbk = np.broadcast_to(kb_keys.reshape(KB_LEN, H, HD).transpose(1, 0, 2)[None],
                          (B, H, KB_LEN, HD))
    kbv = np.broadcast_to(kb_values.reshape(KB_LEN, H, HD).transpose(1, 0, 2)[None],
                          (B, H, KB_LEN, HD))
    aw2 = np.einsum('bhqd,bhkd->bhqk', q2, kbk) * SCALE
    scores = aw2.sum((1, 2))
    idx = np.argsort(-scores, axis=-1)[:, :TOPK]
    kbk = np.take_along_axis(kbk, idx[:, None, :, None], axis=2)
    kbv = np.take_along_axis(kbv, idx[:, None, :, None], axis=2)
    k_all = np.concatenate([np.ascontiguousarray(kbk), k], axis=2)
    v_all = np.concatenate([np.ascontiguousarray(kbv), v], axis=2)
    pad_mask = np.all(attention_mask < 0, axis=-1, keepdims=True)
    kb_mask = np.where(pad_mask, PAD, 0.0).astype(np.float32)
    mask = np.concatenate([np.broadcast_to(kb_mask, (B, 1, Qn, TOPK)),
                           attention_mask], axis=-1)
    attn = np.einsum('bhqd,bhkd->bhqk', q, k_all) * SCALE
    attn[..., :TOPK] += score_shift.reshape(1, H, 1, 1)
    attn = attn + mask
    attn = attn - attn.max(-1, keepdims=True)
    attn = np.exp(attn)
    attn /= attn.sum(-1, keepdims=True)
    out = np.einsum('bhqk,bhkd->bhqd', attn, v_all)
    out = out.transpose(0, 2, 1, 3).reshape(B, Qn, H * HD)
    return out @ Wo.T


def kernel(hidden_states, attention_mask, position_ids, kb_keys, kb_values,
           Wq, Wq2, Wk, Wv, Wo, score_shift):
    hidden_states = np.asarray(hidden_states, np.float32)
    attention_mask = np.asarray(attention_mask, np.float32)
    Wq = np.asarray(Wq, np.float32)
    Wq2 = np.asarray(Wq2, np.float32)
    Wk = np.asarray(Wk, np.float32)
    Wv = np.asarray(Wv, np.float32)
    Wo = np.asarray(Wo, np.float32)
    kb_keys = np.asarray(kb_keys, np.float32)
    kb_values = np.asarray(kb_values, np.float32)
    score_shift = np.asarray(score_shift, np.float32)

    causal = np.where(np.tril(np.ones((Q, Q), bool)), 0.0, PAD).astype(np.float32)
    if (hidden_states.shape != (1, Q, HID)
            or attention_mask.shape != (1, 1, Q, Q)
            or not np.array_equal(attention_mask[0, 0], causal)):
        return _numpy_fallback(hidden_states, attention_mask, position_ids,
                               kb_keys, kb_values, Wq, Wq2, Wk, Wv, Wo,
                               score_shift).astype(np.float32)

    global _CACHED_NC
    if _CACHED_NC is None:
        _CACHED_NC = build()
    nc = _CACHED_NC

    in_maps = _host_prep(hidden_states, position_ids, kb_keys, kb_values,
                         Wq, Wq2, Wk, Wv, Wo, score_shift)
    res = bass_utils.run_bass_kernel_spmd(nc, in_maps, core_ids=list(range(N_CORES)))
    outT = res.results[0]["outT"].astype(np.float64)
    for c in range(1, N_CORES):
        outT += res.results[c]["outT"]
    return np.ascontiguousarray(outT.T).reshape(1, Q, HID).astype(np.float32)
